# revision 64
# baseline (speedup 1.0000x reference)
"""Trainium2 Bass kernel for CDRExtractor (segment_reduce).

Input : segmentation_mask (64, 3, 512, 512) fp32
Output: (64, 5) fp32 = [cdr, disc_mean, cup_mean, disc_mean, cup_mean]

Sharding: pure data parallel, 8 samples per core across 8 cores; each core
streams its 24 MiB shard once.

v4 design (72.5us vs the 95.0us baseline, CoreSim cost model):
  - The input DMA (75.8us of modeled transfer cost) is split across
    THREE DGE queues (SP ~75%, Pool ~20%, ACT fill-phase only) so it
    overlaps with itself.
  - t-space d-tests (argmax row-presence) WITHOUT exp: margin mins
    m_l = min(t_l, t_l - t_other) via DVE TT (2x uop), counted by
    tensor_scalar is_gt/add with fused row-accum (4x uop; op1=add is
    the one accum variant CoreSim executes).
  - softmax sums: only p2 = f2*r is materialized; plane sums of r (=p0)
    and p2 accumulate via PE one-hot matmuls into two PSUM banks
    [8, 512] (PE is otherwise idle), and cup_mean = 1 - p0m - p2m.
  - r = 1/(1+f1+f2) via ACT ln(bias=1)+exp(-L); one fat tile uses
    Pool(+1) + DVE reciprocal to balance the ACT and DVE spines.
  - 4-sample fat tiles amortize the 185ns ACT instruction init; 2-sample
    tiles taper both ends of the schedule (pipeline fill/drain).
  - tail: PS0 sum via ACT copy-accum, PS2 via DVE reduce (critical-path
    driven split); d-tail penalty/iota trick as in the baseline.
  - critical-path-driven micro-placement: oh const on the ACT queue,
    recip-tile +1 add on DVE (4x), per-sample end products. NB Pool
    reading PSUM fails the real axon lowering (pen stays on DVE).

Critical path (from the perfetto trace): Pool fill -> DVE spine
(u/v/m/d + sadd + 1 reciprocal) -> last products -> PE matmuls ->
PS2 reduce -> out DMA. End-to-end 72.5us. HW-verified rel err
2.3e-04 (tol 2e-2).
"""

import numpy as np
from contextlib import ExitStack

B, C, H, W = 64, 3, 512, 512
NCORES = 8
SPC = B // NCORES      # samples per core = 8
NB = H // 128          # 128-row blocks = 4
HW = float(H * W)

_CACHE = {}

# schedule: (s0, ns, b) tiles; tapered at both ends
TILES = [
    (0, 2, 0), (2, 2, 0), (4, 4, 0),
    (0, 4, 1), (4, 4, 1),
    (0, 4, 2), (4, 4, 2),
    (0, 4, 3), (4, 2, 3), (6, 2, 3),
]
P2_DVE_NS = 0      # leading samples of p2 on DVE for fat tiles
EXTRA_UNIT = "act"  # who gets the 12th dma unit on even fat tiles
SPLIT_EXP = False   # exp in 2-sample halves
SPLIT_SUB = True   # subs in 2-sample halves
END_RR = False      # last two tiles round-robin their dma chunks
FILL_RR = 3         # tiles with index < FILL_RR round-robin their dma chunks
MS1_ON_ACT = False  # (unused)
POOL_UNITS = 3      # pool dma units per fat tile (excl. EXTRA)
SADD_POOL_NS = 1    # trailing samples of sadd on Pool for fat tiles
FILL_ACT = True     # include ACT in fill-phase dma round-robin
MS2_DVE = True     # PS2 reduce on DVE instead of ACT copy-accum
RECIP_TILES = (3,)    # tiles whose r comes from DVE reciprocal (ACT relief)
D_MPATH = True     # d-counts via margin mins + 4x ts-accum
SPLIT_SUB2 = True  # ns=2 subs split per sample
MPATH_SPLIT_FIRST = 0  # tiles < this get per-sample m-path ops (fill)
T_BUFS = 3
SPLIT_EXP2 = 0      # tiles < this get per-sample exp instructions
FILL_HALF = False   # fill-phase dma in half-width chunks
FILL_ORDER = "sap"  # fill-phase queue rotation (s=sp, a=act, p=pool)
PE_WARM = 0         # dummy matmuls per tile to build the PE p-state ramp
MS0_ACT = True     # PS0 reduce via ACT copy-accum (off the DVE tail)
PEN_POOL = False    # d-tail penalty op on Pool instead of DVE
O_ACT = False       # O mean-rows via ACT scale-copy instead of DVE ts
PROD_SPLIT_END = True  # per-sample products on the last two tiles
OH_Q = "act"       # queue for the oh const load
RECIP_ADD_DVE = True  # recip-tile +1 add on DVE ts (4x) instead of Pool
RECIP_HALF = ()     # fat tiles whose samples 2:4 use the DVE recip path
LN_PAIRS = {}       # ns2-tile pairs {first: second} sharing one wide ln/rexp
EXTRA_TILES = None  # explicit tile set for the EXTRA dma unit (None = even)
PROD_END_DVE = False  # last-tile products on DVE instead of Pool
WARM_LATE = False   # emit warm act + oh after tile-0 dma (fill overlap)
F_BUFS = 3
RB_BUFS = 2
SA_BUFS = 3
UV_BUFS = 1
LN_BUFS = 2


def _build():
    import concourse.bass as bass
    import concourse.bacc as bacc
    import concourse.mybir as mybir
    from concourse.tile import TileContext

    # Offer only the act-table set containing BOTH exp and ln so the act
    # table never reloads mid-kernel.
    if not _CACHE.get("act_patch"):
        _orig_tables = bacc.get_activation_tables

        def _only_ln_exp(arch):
            t = _orig_tables(arch)
            keep = "natural_log_exp_and_others"
            return {k: (v if k == keep else set()) for k, v in t.items()}

        bacc.get_activation_tables = _only_ln_exp
        _CACHE["act_patch"] = True

    f32 = mybir.dt.float32
    bf16 = mybir.dt.bfloat16
    Alu = mybir.AluOpType
    AFT = mybir.ActivationFunctionType
    X_AX = mybir.AxisListType.X

    nc = bacc.Bacc()
    x = nc.dram_tensor("x", (SPC, C, H, W), f32, kind="ExternalInput")
    iota_in = nc.dram_tensor("iota", (32, 128), f32, kind="ExternalInput")
    ident_in = nc.dram_tensor("ident", (128, 128), f32, kind="ExternalInput")
    oh_in = nc.dram_tensor("oh", (128, 64), bf16, kind="ExternalInput")
    out = nc.dram_tensor("out", (5, SPC), f32, kind="ExternalOutput")

    with TileContext(nc) as tc, ExitStack() as ctx:
        cpool = ctx.enter_context(tc.tile_pool(name="consts", bufs=1))
        apool = ctx.enter_context(tc.tile_pool(name="accs", bufs=1))
        mpool = ctx.enter_context(tc.tile_pool(name="main", bufs=2))
        ppool = ctx.enter_context(tc.tile_pool(name="ps", bufs=1, space="PSUM"))

        # dummy activation on a memset tile: forces the (one-time) act
        # table load to run at t~0 instead of behind the first X DMA
        warm = cpool.tile([1, 16], bf16, tag="warm")
        nc.vector.memset(warm[:, :], 0.0)
        if not WARM_LATE:
            nc.scalar.activation(warm[:, :], warm[:, :], AFT.Exp)

        iota = cpool.tile([32, 128], f32, tag="iota")
        ident = cpool.tile([128, 128], f32, tag="ident")
        oh = cpool.tile([128, 64], bf16, tag="oh")
        if not WARM_LATE:
            # one-hot stationaries needed from the first matmul on
            {"pool": nc.gpsimd, "act": nc.scalar,
             "sp": nc.sync}[OH_Q].dma_start(oh[:, :], oh_in[:, :])

        # accumulators: col j = b*8 + s
        DM1 = apool.tile([128, 32], f32, tag="DM1")  # row argmax counts lbl1
        DM2 = apool.tile([128, 32], f32, tag="DM2")
        # psum accumulators for plane sums: row = sample
        PS0 = ppool.tile([8, 512], f32, tag="PS0")
        PS2 = ppool.tile([8, 512], f32, tag="PS2")
        PSW = None
        if PE_WARM:
            PSW = ppool.tile([8, 512], f32, tag="PSW", name="PSW")
        DVE_P2 = {4, 6}
        mm_count = [0]
        MM_TOTAL = sum(ns for (_, ns, _) in TILES) * 2

        # per-tile DMA chunk assignment. SP carries ~80% of the bytes;
        # Pool absorbs ~2 units/fat tile; ACT only helps during the fill
        # (it is the busiest engine otherwise).
        def dma_chunks(i, s0, ns, b):
            units = [(s, c) for s in range(s0, s0 + ns) for c in range(C)]
            if i < FILL_RR or (END_RR and i >= len(TILES) - 2):
                # fill/drain phase: all queues in parallel
                qmap = {"s": "sp", "a": "act", "p": "pool"}
                order = [qmap[ch] for ch in FILL_ORDER]
                if not FILL_ACT:
                    order = [q for q in order if q != "act"] or ["sp"]
                if FILL_HALF:
                    half = [(s, c, hw) for (s, c) in units for hw in (0, 1)]
                    return [(order[j % len(order)], s, c, hw)
                            for j, (s, c, hw) in enumerate(half)]
                return [(order[j % len(order)], s, c)
                        for j, (s, c) in enumerate(units)]
            if ns == 4:
                on = (i in EXTRA_TILES) if EXTRA_TILES is not None \
                    else (i % 2 == 0)
                last = EXTRA_UNIT if on else "sp"
                npool = POOL_UNITS
                nsp = 11 - npool
                qs = []
                for j in range(11):
                    if npool and j % (11 // npool + 1) == (11 // npool):
                        qs.append("pool")
                        npool -= 1
                    else:
                        qs.append("sp")
                qs.append(last)
            elif ns == 2:
                qs = ["sp"] * 4 + ["pool", "sp"]
            else:
                qs = ["sp"] * 3
            return [(q, s, c) for (s, c), q in zip(units, qs)]

        def stage_dma(i):
            s0, ns, b = TILES[i]
            Xf = mpool.tile([128, 4 * C * W], f32, tag="X",
                            name=f"X_{i}", bufs=4)
            X = Xf[:, 0:ns * C * W]
            Xv = X.rearrange("p (s c w) -> p s c w", s=ns, c=C)
            for ch in dma_chunks(i, s0, ns, b):
                if len(ch) == 4:
                    q, s, c, hw = ch
                    src = x[s, c, b * 128:(b + 1) * 128,
                            hw * 256:(hw + 1) * 256]
                    dst = Xv[:, s - s0, c, hw * 256:(hw + 1) * 256]
                else:
                    q, s, c = ch
                    src = x[s, c, b * 128:(b + 1) * 128, :]
                    dst = Xv[:, s - s0, c, :]
                if q == "sp":
                    nc.sync.dma_start(dst, src)
                elif q == "act":
                    nc.scalar.dma_start(dst, src)
                else:
                    nc.gpsimd.dma_start(dst, src)
            return X

        def stage_sub(i, X):
            """Pool: T = [x1-x0 | x2-x0] in bf16, laid out (s, l, w)."""
            s0, ns, b = TILES[i]
            Xv = X.rearrange("p (s c w) -> p s c w", s=ns, c=C)
            Tf = mpool.tile([128, 4 * 2 * W], bf16, tag="T",
                            name=f"T_{i}", bufs=T_BUFS)
            T = Tf[:, 0:ns * 2 * W]
            Tv = T.rearrange("p (s l w) -> p s l w", s=ns, l=2)
            if SPLIT_SUB and ns == 4:
                for h in (slice(0, 2), slice(2, 4)):
                    for li in range(2):
                        nc.gpsimd.tensor_tensor(
                            Tv[:, h, li, :], Xv[:, h, li + 1, :],
                            Xv[:, h, 0, :], Alu.subtract)
            elif SPLIT_SUB2 and ns == 2:
                for h in (slice(0, 1), slice(1, 2)):
                    for li in range(2):
                        nc.gpsimd.tensor_tensor(
                            Tv[:, h, li, :], Xv[:, h, li + 1, :],
                            Xv[:, h, 0, :], Alu.subtract)
            else:
                for li in range(2):
                    nc.gpsimd.tensor_tensor(
                        Tv[:, :, li, :], Xv[:, :, li + 1, :], Xv[:, :, 0, :],
                        Alu.subtract)
            return T

        def stage_exp(i, T):
            """ACT: F = exp(T) bf16."""
            s0, ns, b = TILES[i]
            Ff = mpool.tile([128, 4 * 2 * W], bf16, tag="F",
                            name=f"F_{i}", bufs=F_BUFS)
            F = Ff[:, 0:ns * 2 * W]
            if i < SPLIT_EXP2:
                h = 2 * W
                for si in range(ns):
                    nc.scalar.activation(F[:, si * h:(si + 1) * h],
                                         T[:, si * h:(si + 1) * h], AFT.Exp)
            elif SPLIT_EXP and ns == 4:
                h = ns * W
                nc.scalar.activation(F[:, 0:h], T[:, 0:h], AFT.Exp)
                nc.scalar.activation(F[:, h:2 * h], T[:, h:2 * h], AFT.Exp)
            else:
                nc.scalar.activation(F[:, :], T[:, :], AFT.Exp)
            return F

        pairbuf = {}

        def stage_sadd(i, F):
            """DVE: sadd = f1 + f2 (bf16, 2x uop). ns2 LN_PAIRS share one
            buffer so a single wide ln/rexp can span both tiles."""
            s0, ns, b = TILES[i]
            Fv = F.rearrange("p (s l w) -> p s l w", s=ns, l=2)
            if i in LN_PAIRS:
                sf = mpool.tile([128, 4 * W], bf16, tag="sa",
                                name=f"sa_{i}", bufs=SA_BUFS)
                pairbuf[i] = sf
                sadd = sf[:, 0:ns * W]
            elif i in LN_PAIRS.values():
                j = [a for a, bb in LN_PAIRS.items() if bb == i][0]
                sf = pairbuf[j]
                nsa = TILES[j][1]
                sadd = sf[:, nsa * W:(nsa + ns) * W]
            else:
                sf = mpool.tile([128, 4 * W], bf16, tag="sa",
                                name=f"sa_{i}", bufs=SA_BUFS)
                sadd = sf[:, 0:ns * W]
            sv = sadd.rearrange("p (s w) -> p s w", s=ns)
            k = ns - SADD_POOL_NS if ns == 4 else ns
            if k > 0:
                nc.vector.tensor_tensor(sv[:, 0:k, :], Fv[:, 0:k, 0, :],
                                        Fv[:, 0:k, 1, :], Alu.add)
            if k < ns:
                nc.gpsimd.tensor_tensor(sv[:, k:ns, :], Fv[:, k:ns, 0, :],
                                        Fv[:, k:ns, 1, :], Alu.add)
            return sadd

        def stage_mm(i, T):
            """DVE: either MM = max(t_other, 0) (4x), or margin mins
            m_l = min(t_l, t_l - t_other) for the m-path."""
            s0, ns, b = TILES[i]
            Tv = T.rearrange("p (s l w) -> p s l w", s=ns, l=2)
            MMf = mpool.tile([128, 4 * 2 * W], bf16, tag="MM",
                             name=f"MM_{i}", bufs=1)
            MM = MMf[:, 0:ns * 2 * W]
            MMv = MM.rearrange("p (l s w) -> p l s w", l=2, s=ns)
            if D_MPATH:
                UVf = mpool.tile([128, 4 * 2 * W], bf16, tag="UV",
                                 name=f"UV_{i}", bufs=UV_BUFS)
                u = UVf[:, 0:ns * W]
                v = UVf[:, 4 * W:4 * W + ns * W]
                uv = u.rearrange("p (s w) -> p s w", s=ns)
                vv = v.rearrange("p (s w) -> p s w", s=ns)
                if i < MPATH_SPLIT_FIRST:
                    for si in range(ns):
                        ssl = slice(si, si + 1)
                        nc.vector.tensor_tensor(uv[:, ssl, :],
                                                Tv[:, ssl, 0, :],
                                                Tv[:, ssl, 1, :],
                                                Alu.subtract)
                        nc.vector.tensor_scalar_mul(vv[:, ssl, :],
                                                    uv[:, ssl, :], -1.0)
                        nc.vector.tensor_tensor(MMv[:, 0, ssl, :],
                                                Tv[:, ssl, 0, :],
                                                uv[:, ssl, :], Alu.min)
                        nc.vector.tensor_tensor(MMv[:, 1, ssl, :],
                                                Tv[:, ssl, 1, :],
                                                vv[:, ssl, :], Alu.min)
                else:
                    nc.vector.tensor_tensor(uv[:, :, :], Tv[:, :, 0, :],
                                            Tv[:, :, 1, :], Alu.subtract)
                    nc.vector.tensor_scalar_mul(v[:, :], u[:, :], -1.0)
                    nc.vector.tensor_tensor(MMv[:, 0, :, :], Tv[:, :, 0, :],
                                            uv[:, :, :], Alu.min)
                    nc.vector.tensor_tensor(MMv[:, 1, :, :], Tv[:, :, 1, :],
                                            vv[:, :, :], Alu.min)
            else:
                nc.vector.tensor_scalar(MMv[:, 0, :, :], Tv[:, :, 1, :], 0.0,
                                        None, Alu.max)
                nc.vector.tensor_scalar(MMv[:, 1, :, :], Tv[:, :, 0, :], 0.0,
                                        None, Alu.max)
            return MM

        def stage_d(i, T, MM):
            """DVE: d-counts. MM-path: STT is_gt w/ accum (1x).
            m-path: tensor_scalar is_gt/add w/ accum (4x)."""
            s0, ns, b = TILES[i]
            Tv = T.rearrange("p (s l w) -> p s l w", s=ns, l=2)
            MMv = MM.rearrange("p (l s w) -> p l s w", l=2, s=ns)
            df = mpool.tile([128, 4 * 2 * W], bf16, tag="UV",
                            name=f"dsv_{i}", bufs=UV_BUFS)
            dscr = df[:, 0:ns * 2 * W]
            dv = dscr.rearrange("p (s l w) -> p s l w", s=ns, l=2)
            for si in range(ns):
                col = b * 8 + (s0 + si)
                for li, DM in ((0, DM1), (1, DM2)):
                    if D_MPATH:
                        nc.vector.tensor_scalar(
                            dv[:, si, li, :], MMv[:, li, si, :], 0.0, 0.0,
                            Alu.is_gt, Alu.add,
                            accum_out=DM[:, col:col + 1])
                    else:
                        nc.vector.scalar_tensor_tensor(
                            dv[:, si, li, :], Tv[:, si, li, :], 0.0,
                            MMv[:, li, si, :], Alu.add, Alu.is_gt,
                            accum_out=DM[:, col:col + 1])

        pair_rb = {}

        def stage_lnrexp(i, sadd):
            """r = 1/(1+f1+f2): ACT ln+exp, or DVE recip for RECIP_TILES.
            LN_PAIRS: the first member emits one wide ln/rexp for both."""
            s0, ns, b = TILES[i]
            if i in pair_rb:
                return pair_rb.pop(i)
            if i in LN_PAIRS:
                j = LN_PAIRS[i]
                nsj = TILES[j][1]
                wid = (ns + nsj) * W
                sf = pairbuf[i]
                lf = mpool.tile([128, 4 * W], bf16, tag="ln",
                                name=f"ln_{i}", bufs=LN_BUFS)
                rf = mpool.tile([128, 4 * W], bf16, tag="rb",
                                name=f"rb_{i}", bufs=RB_BUFS)
                nc.scalar.activation(lf[:, 0:wid], sf[:, 0:wid], AFT.Ln,
                                     bias=1.0)
                nc.scalar.activation(rf[:, 0:wid], lf[:, 0:wid], AFT.Exp,
                                     scale=-1.0)
                pair_rb[j] = rf[:, ns * W:wid]
                return rf[:, 0:ns * W]
            lf = mpool.tile([128, 4 * W], bf16, tag="ln",
                            name=f"ln_{i}", bufs=LN_BUFS)
            lns = lf[:, 0:ns * W]
            rf = mpool.tile([128, 4 * W], bf16, tag="rb",
                            name=f"rb_{i}", bufs=RB_BUFS)
            rb = rf[:, 0:ns * W]
            if i in RECIP_TILES:
                # +1 add: Pool or DVE (4x bf16)
                eng1 = nc.vector if RECIP_ADD_DVE else nc.gpsimd
                eng1.tensor_scalar_add(lns[:, :], sadd[:, :], 1.0)
                with nc.allow_low_precision(reason="bf16 softmax r"):
                    nc.vector.reciprocal(rb[:, :], lns[:, :])
            elif i in RECIP_HALF and ns == 4:
                h = 2 * W
                nc.scalar.activation(lns[:, 0:h], sadd[:, 0:h], AFT.Ln,
                                     bias=1.0)
                nc.scalar.activation(rb[:, 0:h], lns[:, 0:h], AFT.Exp,
                                     scale=-1.0)
                nc.vector.tensor_scalar_add(lns[:, h:2 * h],
                                            sadd[:, h:2 * h], 1.0)
                with nc.allow_low_precision(reason="bf16 softmax r"):
                    nc.vector.reciprocal(rb[:, h:2 * h], lns[:, h:2 * h])
            else:
                nc.scalar.activation(lns[:, :], sadd[:, :], AFT.Ln, bias=1.0)
                nc.scalar.activation(rb[:, :], lns[:, :], AFT.Exp, scale=-1.0)
            return rb

        def stage_prod(i, F, rb):
            """p2 = f2 * r only; p1-sums come from N - sum(r) - sum(p2)."""
            s0, ns, b = TILES[i]
            Fv = F.rearrange("p (s l w) -> p s l w", s=ns, l=2)
            rv = rb.rearrange("p (s w) -> p s w", s=ns)
            pf = mpool.tile([128, 4 * W], bf16, tag="pc",
                            name=f"pc_{i}", bufs=1)
            pscr = pf[:, 0:ns * W]
            pv = pscr.rearrange("p (s w) -> p s w", s=ns)
            if ns == 4:
                k = P2_DVE_NS
                if k > 0:
                    nc.vector.tensor_tensor(pv[:, 0:k, :],
                                            Fv[:, 0:k, 1, :],
                                            rv[:, 0:k, :], Alu.mult)
                nc.gpsimd.tensor_tensor(pv[:, k:4, :], Fv[:, k:4, 1, :],
                                        rv[:, k:4, :], Alu.mult)
            elif PROD_SPLIT_END and i >= len(TILES) - 2:
                peng = nc.vector if PROD_END_DVE else nc.gpsimd
                for si in range(ns):
                    ssl = slice(si, si + 1)
                    peng.tensor_tensor(pv[:, ssl, :], Fv[:, ssl, 1, :],
                                       rv[:, ssl, :], Alu.mult)
            else:
                nc.vector.tensor_tensor(pv[:, :, :], Fv[:, :, 1, :],
                                        rv[:, :, :], Alu.mult)
            return pscr

        def stage_pe(i, rb, pscr):
            """PE: one-hot matmuls accumulate plane sums into PSUM.
            PS0 accumulates r (= p0), PS2 accumulates p2. Optional dummy
            matmuls (gated on rb) pre-build the PE p-state ramp so the
            real matmuls run at full clock."""
            s0, ns, b = TILES[i]
            for wi in range(PE_WARM):
                nc.tensor.matmul(PSW[:, :], oh[:, 0:8], rb[:, 0:512],
                                 start=True, stop=True)
            pv = pscr.rearrange("p (s w) -> p s w", s=ns)
            rv = rb.rearrange("p (s w) -> p s w", s=ns)
            for si in range(ns):
                s = s0 + si
                for src_v, PS in ((rv, PS0), (pv, PS2)):
                    k = mm_count[0]
                    mm_count[0] += 1
                    nc.tensor.matmul(PS[:, :], oh[:, 8 * s:8 * s + 8],
                                     src_v[:, si, :],
                                     start=(k < 2), stop=(k >= MM_TOTAL - 2))

        # ---- software-pipelined emission (2-tile skew) ----
        # ACT order per iteration: [ln/rexp for i-2, exp for i] so the
        # ln of tile i runs two iterations later than its sadd (no ACT
        # stall on the DVE round trip); products likewise at i-2.
        N = len(TILES)
        Xs = {}
        state = {}   # i -> (F, sadd)

        def drain_stage(j):
            Fm, saddm = state.pop(j)
            rbm = stage_lnrexp(j, saddm)
            pscr = stage_prod(j, Fm, rbm)
            stage_pe(j, rbm, pscr)

        for i in range(N):
            if i == 0:
                Xs[0] = stage_dma(0)
                Xs[1] = stage_dma(1)
                if WARM_LATE:
                    nc.scalar.activation(warm[:, :], warm[:, :], AFT.Exp)
                    {"pool": nc.gpsimd, "act": nc.scalar,
                     "sp": nc.sync}[OH_Q].dma_start(oh[:, :], oh_in[:, :])
            T = stage_sub(i, Xs[i])
            if i >= 2:
                drain_stage(i - 2)
            F = stage_exp(i, T)
            MM = stage_mm(i, T)
            sadd = stage_sadd(i, F)
            stage_d(i, T, MM)
            if i + 2 < N:
                Xs[i + 2] = stage_dma(i + 2)
            state[i] = (F, sadd)
        drain_stage(N - 2)
        drain_stage(N - 1)

        # ---- tail ----
        # const loads for the d-tail (deferred; they aren't needed earlier)
        nc.sync.dma_start(iota[:, :], iota_in[:, :])
        nc.sync.dma_start(ident[:, :], ident_in[:, :])
        O = cpool.tile([1, 40], f32, tag="O")

        # p-tail: PSUM [8,512] -> [8,1] sums (DVE + ACT in parallel), then
        # transpose to [1,8]
        ms0 = cpool.tile([8, 1], f32, tag="ms0")
        ms2 = cpool.tile([8, 1], f32, tag="ms2")
        msum = cpool.tile([8, 1], f32, tag="msum")
        ms2scr = cpool.tile([8, 512], f32, tag="ms2scr")
        if MS0_ACT:
            ms0scr = cpool.tile([8, 512], f32, tag="ms0scr")
            nc.scalar.activation(ms0scr[:, :], PS0[:, :], AFT.Copy,
                                 accum_out=ms0[:, :])
        else:
            nc.vector.tensor_reduce(ms0[:, :], PS0[:, :], X_AX, op=Alu.add)
        if MS2_DVE:
            nc.vector.tensor_reduce(ms2[:, :], PS2[:, :], X_AX, op=Alu.add)
        else:
            nc.scalar.activation(ms2scr[:, :], PS2[:, :], AFT.Copy,
                                 accum_out=ms2[:, :])
        nc.vector.tensor_tensor(msum[:, :], ms0[:, :], ms2[:, :], Alu.add)
        MT = ppool.tile([1, 16], f32, tag="MT")
        nc.tensor.transpose(MT[:, 0:8], msum[:, :], ident[0:8, 0:8])
        nc.tensor.transpose(MT[:, 8:16], ms2[:, :], ident[0:8, 0:8])
        sc = 1.0 / HW
        # rows: [cdr, disc=l2, cup=l1, disc, cup]; cup = 1 - (p0m + p2m)
        if O_ACT:
            nc.scalar.activation(O[:, 8:16], MT[:, 8:16], AFT.Copy, scale=sc)
            nc.scalar.activation(O[:, 16:24], MT[:, 0:8], AFT.Copy,
                                 scale=-sc, bias=1.0)
            nc.scalar.activation(O[:, 24:32], MT[:, 8:16], AFT.Copy, scale=sc)
            nc.scalar.activation(O[:, 32:40], MT[:, 0:8], AFT.Copy,
                                 scale=-sc, bias=1.0)
        else:
            nc.vector.tensor_scalar(O[:, 8:16], MT[:, 8:16], sc, None,
                                    Alu.mult)
            nc.vector.tensor_scalar(O[:, 16:24], MT[:, 0:8], -sc, 1.0,
                                    Alu.mult, Alu.add)
            nc.vector.tensor_scalar(O[:, 24:32], MT[:, 8:16], sc, None,
                                    Alu.mult)
            nc.vector.tensor_scalar(O[:, 32:40], MT[:, 0:8], -sc, 1.0,
                                    Alu.mult, Alu.add)

        # d-tail: heights from DM1/DM2 (as in baseline)
        heights = []
        dparts = []
        for li, DM in enumerate((DM1, DM2)):
            TD = ppool.tile([32, 128], f32, tag="TD", name=f"TD{li}")
            nc.tensor.transpose(TD[:, :], DM[:, :], ident[:, :])
            pen = cpool.tile([32, 128], f32, tag=f"pen{li}")
            peng = nc.gpsimd if PEN_POOL else nc.vector
            peng.tensor_scalar(pen[:, :], TD[:, :], 0.5, 1e6,
                               Alu.is_lt, Alu.mult)
            cmin = cpool.tile([32, 128], f32, tag=f"cmin{li}")
            nc.gpsimd.tensor_tensor(cmin[:, :], pen[:, :], iota[:, :], Alu.add)
            cmax = cpool.tile([32, 128], f32, tag=f"cmax{li}")
            nc.gpsimd.tensor_tensor(cmax[:, :], iota[:, :], pen[:, :],
                                    Alu.subtract)
            Y = cpool.tile([32, 2], f32, tag=f"Y{li}")
            nc.vector.tensor_reduce(Y[:, 0:1], cmin[:, :], X_AX, op=Alu.min)
            nc.vector.tensor_reduce(Y[:, 1:2], cmax[:, :], X_AX, op=Alu.max)
            YTmin = ppool.tile([1, 32], f32, tag="YTmin", name=f"YTmin{li}")
            YTmax = ppool.tile([1, 32], f32, tag="YTmax", name=f"YTmax{li}")
            nc.tensor.transpose(YTmin[:, :], Y[:, 0:1], ident[0:32, 0:32])
            nc.tensor.transpose(YTmax[:, :], Y[:, 1:2], ident[0:32, 0:32])
            ymin8 = cpool.tile([1, 8], f32, tag=f"ymin{li}")
            ymax8 = cpool.tile([1, 8], f32, tag=f"ymax{li}")
            nc.vector.tensor_reduce(
                ymin8[:, :], YTmin[0:1, :].rearrange("p (b s) -> p s b", b=4),
                X_AX, op=Alu.min)
            nc.vector.tensor_reduce(
                ymax8[:, :], YTmax[0:1, :].rearrange("p (b s) -> p s b", b=4),
                X_AX, op=Alu.max)
            hL = cpool.tile([1, 8], f32, tag=f"h{li}")
            nc.vector.tensor_tensor(hL[:, :], ymax8[:, :], ymin8[:, :],
                                    Alu.subtract)
            nc.vector.tensor_scalar_max(hL[:, :], hL[:, :], 0.0)
            heights.append(hL)

        h_cup, h_disc = heights
        den = cpool.tile([1, 8], f32, tag="den")
        nc.vector.tensor_scalar_add(den[:, :], h_disc[:, :], 1e-6)
        rec = cpool.tile([1, 8], f32, tag="rec")
        nc.vector.reciprocal(rec[:, :], den[:, :])
        nc.vector.tensor_tensor(O[:, 0:8], h_cup[:, :], rec[:, :], Alu.mult)

        nc.sync.dma_start(out[:, :], O[:, :])

    nc.finalize()
    return nc


def _get_nc():
    if "nc" not in _CACHE:
        _CACHE["nc"] = _build()
    return _CACHE["nc"]


def _host_inputs():
    iota = (np.arange(128, dtype=np.float32)[None, :]
            + 128.0 * np.repeat(np.arange(4, dtype=np.float32), 8)[:, None])
    ident = np.eye(128, dtype=np.float32)
    # oh[:, 8s+j] = 1 iff j == s (one-hot stationary for per-sample matmul)
    import ml_dtypes
    oh = np.zeros((128, 64), dtype=ml_dtypes.bfloat16)
    for s in range(8):
        oh[:, 8 * s + s] = 1.0
    return iota, ident, oh


def _run(seg_mask, trace=False):
    from concourse.bass_utils import run_bass_kernel_spmd

    x = np.ascontiguousarray(np.asarray(seg_mask, dtype=np.float32))
    assert x.shape == (B, C, H, W)
    iota, ident, oh = _host_inputs()
    in_maps = [
        {"x": x[SPC * c:SPC * (c + 1)], "iota": iota, "ident": ident,
         "oh": oh}
        for c in range(NCORES)
    ]
    nc = _get_nc()
    res = run_bass_kernel_spmd(nc, in_maps, core_ids=list(range(NCORES)),
                               trace=trace)
    outs = []
    for c in range(NCORES):
        o = np.asarray(res.results[c]["out"]).reshape(5, SPC).T
        outs.append(o)
    full = np.concatenate(outs, axis=0).astype(np.float32)
    return full, res


def kernel(segmentation_mask):
    full, _ = _run(segmentation_mask, trace=False)
    return full
